# revision 25
# baseline (speedup 1.0000x reference)
"""GCN encoder kernel for 8 Trainium2 NeuronCores.

Math: out = A_hat @ (x @ (W_gc @ W_fc)) + (b_gc @ W_fc + b_fc), with
A_hat = D^-1/2 (A + I) D^-1/2 (degree over destinations incl self-loops).

Factorization: out[d] = dinv[d] * (sum_{e: dst=d} ghat[src_e] + ghat[d]) + b_out
with ghat[s] = dinv[s] * (x[s] @ W2), W2 = W_gc @ W_fc. So no per-edge norm
is needed on device: the one-hot segmented-sum matmul uses exact 0/1 weights.

Distribution (1D graph parallel, dst-partitioned):
  - nodes sharded 8 ways (12544-padded); each core computes its shard of
    ghat (bf16) from host-pretransposed x, AllGather -> full bf16 table,
  - per core, edges with local destination are bucketed by
    (dst tile, src pair-group, src parity); gather elements are PAIRS of
    bf16 rows (256 B) so indices fit int16 with only 2 groups; the matmul
    rhs selects the 64-col half by parity (bucket-constant),
  - buckets are packed back-to-back (padded only to the max count over the
    8 cores, for the SPMD-common schedule); chunks that straddle a tile
    boundary get a second compacted one-hot (ohB) and an extra matmul,
  - self-loop contribution + bias is folded into s0b = dinv^2*g + b_out,
    added at flush: out = dinv*acc + s0b.
"""
import numpy as np
import ml_dtypes
from contextlib import ExitStack

N_NODES = 100000
IN_FEAT = 256
OUT_FEAT = 64
NCORES = 8
SHARD = N_NODES // NCORES          # 12500
NTILES = 98                        # ceil(12500/128)
PADSHARD = NTILES * 128            # 12544
GTAB_ROWS = NCORES * PADSHARD      # 100352
NPAIRS = GTAB_ROWS // 2            # 50176
NGROUPS = 2
GPAIRS = NPAIRS // NGROUPS         # 25088 (< 32768, int16-addressable)
SUPER = 6                          # dst tiles per super
DST_SENTINEL = 255.0               # matches no iota column -> zero one-hot row


def _preprocess(x, edge_index, W_gc, b_gc, W_fc, b_fc):
    """Host-side index/weight preprocessing. Returns the (input-derived,
    core-common) schedule and per-core device arrays."""
    x = np.asarray(x, np.float32)
    W2 = (np.asarray(W_gc, np.float64) @ np.asarray(W_fc, np.float64)).astype(np.float32)
    b_out = (np.asarray(b_gc, np.float64) @ np.asarray(W_fc, np.float64)
             + np.asarray(b_fc, np.float64)).astype(np.float32)

    src = np.asarray(edge_index[0], np.int64)
    dst = np.asarray(edge_index[1], np.int64)
    deg = np.bincount(dst, minlength=N_NODES).astype(np.float64) + 1.0
    dinv = (1.0 / np.sqrt(deg)).astype(np.float32)

    core = dst // SHARD
    dloc = dst % SHARD
    tile_id = dloc // 128
    dst_local = (dloc % 128).astype(np.float32)
    gsrc = (src // SHARD) * PADSHARD + (src % SHARD)
    pairi = gsrc >> 1
    par = (gsrc & 1).astype(np.int64)
    grp = pairi // GPAIRS
    lidx = (pairi % GPAIRS).astype(np.int16)
    gp = grp * 2 + par

    key = (core * NTILES + tile_id) * 4 + gp
    order = np.argsort(key, kind="stable")
    key_s = key[order]
    lidx_s = lidx[order]
    dstl_s = dst_local[order]
    counts = np.bincount(key_s, minlength=NCORES * NTILES * 4).reshape(NCORES, NTILES, 4)
    flat = counts.reshape(NCORES, -1)
    starts = np.zeros_like(counts)
    starts.reshape(NCORES, -1)[:, 1:] = np.cumsum(flat, axis=1)[:, :-1]
    starts += np.concatenate([[0], np.cumsum(flat.sum(axis=1))[:-1]]).reshape(-1, 1, 1)

    mx = np.maximum(counts.max(axis=0), 128)   # [NTILES, 4]; >=128 keeps the
    # "at most one tile starts per chunk" invariant of the packed layout.

    supers = [list(range(i, min(i + SUPER, NTILES))) for i in range(0, NTILES, SUPER)]

    # ---- core-common packed schedule ----
    windows = []           # per (super, gp)
    pieces = {}            # (si, gp) -> {t: [('A', wchunk) | ('B', bcol)]}
    slot_base = {}         # (si, gp, t) -> global slot start
    pos = 0
    for si, tiles in enumerate(supers):
        for g in range(4):
            w0 = pos
            bcols = []
            wp = {}
            for t in tiles:
                at = pos - w0
                slot_base[(si, g, t)] = pos
                m = int(mx[t, g])
                bt = at + m
                c0, c1 = at // 128, (bt - 1) // 128
                pl = []
                if at % 128 != 0:
                    pl.append(("B", len(bcols)))
                    bcols.append(c0)
                    c0 += 1
                pl.extend(("A", c) for c in range(c0, c1 + 1))
                wp[t] = pl
                pos += m
            nreal = pos - w0
            pos = ((pos + 127) // 128) * 128
            windows.append(dict(si=si, gp=g, w0=w0, wch=(pos - w0) // 128,
                                nreal=nreal, nb=len(bcols), bcols=bcols))
            pieces[(si, g)] = wp
    S_total = pos
    C_total = S_total // 128
    boff = 0
    for w in windows:
        w["boff"] = boff
        boff += w["nb"]
    CB_total = max(boff, 1)

    # ---- per-core slot data ----
    idx_all = np.zeros((NCORES, S_total), np.int16)
    dstA_all = np.full((NCORES, S_total), DST_SENTINEL, np.float32)
    dstB_all = np.full((NCORES, CB_total * 128), DST_SENTINEL, np.float32)
    wi = 0
    for si, tiles in enumerate(supers):
        for g in range(4):
            w = windows[wi]
            wi += 1
            wp = pieces[(si, g)]
            for t in tiles:
                a_g = slot_base[(si, g, t)]
                at = a_g - w["w0"]
                lead = (128 - at % 128) % 128
                jcol = w["boff"] + wp[t][0][1] if at % 128 != 0 else -1
                for c in range(NCORES):
                    n = int(counts[c, t, g])
                    if n == 0:
                        continue
                    s0 = int(starts[c, t, g])
                    idx_all[c, a_g:a_g + n] = lidx_s[s0:s0 + n]
                    vals = dstl_s[s0:s0 + n]
                    nlead = min(lead, n)
                    if nlead:
                        p0 = a_g % 128
                        dstB_all[c, jcol * 128 + p0: jcol * 128 + p0 + nlead] = vals[:nlead]
                    if n > nlead:
                        dstA_all[c, a_g + nlead: a_g + n] = vals[nlead:]

    # ---- device layouts ----
    idx_dev = np.ascontiguousarray(
        np.tile(idx_all.reshape(NCORES, -1, 16).transpose(0, 2, 1), (1, 8, 1)))
    dstA_dev = np.ascontiguousarray(
        dstA_all.reshape(NCORES, C_total, 128).transpose(0, 2, 1)).astype(ml_dtypes.bfloat16)
    dstB_dev = np.ascontiguousarray(
        dstB_all.reshape(NCORES, CB_total, 128).transpose(0, 2, 1)).astype(ml_dtypes.bfloat16)

    x_pad = np.zeros((NCORES, PADSHARD, IN_FEAT), np.float32)
    x_pad[:, :SHARD] = x.reshape(NCORES, SHARD, IN_FEAT)
    # [NC, NTILES, p(feat%128), k(feat//128), m(node%128)] - per-tile contiguous
    xT_dev = np.ascontiguousarray(
        x_pad.reshape(NCORES, NTILES, 128, 2, 128).transpose(0, 1, 4, 3, 2))

    dinv_pad = np.zeros((NCORES, PADSHARD), np.float32)
    dinv_pad[:, :SHARD] = dinv.reshape(NCORES, SHARD)
    dinv_dev = np.ascontiguousarray(
        dinv_pad.reshape(NCORES, NTILES, 128).transpose(0, 2, 1))  # [NC, 128, NT]
    dinv2_dev = np.ascontiguousarray(dinv_dev * dinv_dev)

    iota_np = np.tile(np.arange(128, dtype=np.float32)[None, :], (128, 1)).astype(ml_dtypes.bfloat16)
    b8_np = np.tile(b_out[None, :], (128, 1)).astype(np.float32)

    meta = dict(supers=supers, windows=windows, pieces=pieces,
                S_total=S_total, C_total=C_total, CB_total=CB_total)
    per_core = dict(xT=xT_dev, idx=idx_dev, dstA=dstA_dev, dstB=dstB_dev,
                    dinv=dinv_dev, dinv2=dinv2_dev)
    consts = dict(W2=W2, iota=iota_np, b8=b8_np)
    return meta, per_core, consts


def _build(meta):
    import concourse.bass as bass
    import concourse.tile as tile
    from concourse import bacc, mybir

    supers = meta["supers"]
    windows = meta["windows"]
    pieces = meta["pieces"]
    S_total = meta["S_total"]
    C_total = meta["C_total"]
    CB_total = meta["CB_total"]

    nc = bacc.Bacc("TRN2", target_bir_lowering=False, debug=False,
                   num_devices=NCORES, num_swdge_queues=4)
    f32, bf16, i16 = mybir.dt.float32, mybir.dt.bfloat16, mybir.dt.int16
    Copy = mybir.ActivationFunctionType.Copy

    xT_ap = nc.dram_tensor("xt_in", [NTILES, 128, 2, 128], f32, kind="ExternalInput").ap()
    idx_ap = nc.dram_tensor("idx_in", [128, S_total // 16], i16, kind="ExternalInput").ap()
    dstA_ap = nc.dram_tensor("dsta_in", [128, C_total], bf16, kind="ExternalInput").ap()
    dstB_ap = nc.dram_tensor("dstb_in", [128, CB_total], bf16, kind="ExternalInput").ap()
    W2_ap = nc.dram_tensor("w2_in", [IN_FEAT, OUT_FEAT], f32, kind="ExternalInput").ap()
    iota_ap = nc.dram_tensor("iota_in", [128, 128], bf16, kind="ExternalInput").ap()
    dinv_ap = nc.dram_tensor("dinv_in", [128, NTILES], f32, kind="ExternalInput").ap()
    dinv2_ap = nc.dram_tensor("dinv2_in", [128, NTILES], f32, kind="ExternalInput").ap()
    b8_ap = nc.dram_tensor("b8_in", [128, OUT_FEAT], f32, kind="ExternalInput").ap()
    out_ap = nc.dram_tensor("y_out", [128, NTILES, OUT_FEAT], f32, kind="ExternalOutput").ap()

    with tile.TileContext(nc) as tc, ExitStack() as ctx:
        dram = ctx.enter_context(tc.tile_pool(name="dram", bufs=1, space="DRAM"))
        g_c = dram.tile([PADSHARD, OUT_FEAT], bf16)
        g_full = dram.tile([GTAB_ROWS, OUT_FEAT], bf16)

        cpool = ctx.enter_context(tc.tile_pool(name="consts", bufs=1))
        iota_t = cpool.tile([128, 128], bf16)
        nc.sync.dma_start(iota_t[:], iota_ap[:])
        b8_t = cpool.tile([128, OUT_FEAT], f32)
        nc.sync.dma_start(b8_t[:], b8_ap[:])
        dinv_t = cpool.tile([128, NTILES], f32)
        nc.sync.dma_start(dinv_t[:], dinv_ap[:])
        dinv2_t = cpool.tile([128, NTILES], f32)
        nc.sync.dma_start(dinv2_t[:], dinv2_ap[:])
        gl_sb = cpool.tile([128, NTILES, OUT_FEAT], bf16)     # ghat local shard
        s0b = cpool.tile([128, NTILES, OUT_FEAT], bf16)       # dinv^2*g + b_out

        # ---- phase 1: ghat_c = dinv * (x_c @ W2), bf16 ----
        with tc.tile_pool(name="ph1", bufs=3) as ph1, \
             tc.tile_pool(name="ph1c", bufs=1) as ph1c, \
             tc.tile_pool(name="ph1ps", bufs=4, space="PSUM") as ph1ps:
            w2_t = ph1c.tile([128, 2, OUT_FEAT], f32)
            nc.sync.dma_start(w2_t[:], W2_ap.rearrange("(k p) f -> p k f", p=128))
            for nt in range(NTILES):
                xt = ph1.tile([128, 2, 128], f32, tag="xt")
                nc.sync.dma_start(xt[:], xT_ap[nt])
                gp = ph1ps.tile([128, OUT_FEAT], f32, tag="gps")
                for k in range(2):
                    nc.tensor.matmul(gp[:], xt[:, k, :], w2_t[:, k, :],
                                     start=(k == 0), stop=(k == 1))
                nc.vector.tensor_tensor(
                    out=gl_sb[:, nt, :], in0=gp[:],
                    in1=dinv_t[:, nt:nt + 1].broadcast_to([128, OUT_FEAT]),
                    op=mybir.AluOpType.mult)
                nc.sync.dma_start(g_c[nt * 128:(nt + 1) * 128, :], gl_sb[:, nt, :])

        # ---- allgather ghat (bf16) ----
        nc.gpsimd.collective_compute(
            "AllGather", mybir.AluOpType.bypass,
            ins=[g_c.opt()], outs=[g_full.opt()],
            replica_groups=[list(range(NCORES))],
        )
        with tc.tile_pool(name="s0p", bufs=4) as s0p:
            for nt in range(NTILES):
                tmp = s0p.tile([128, OUT_FEAT], f32, tag="s0t")
                nc.scalar.activation(tmp[:], gl_sb[:, nt, :], Copy,
                                     scale=dinv_t[:, nt:nt + 1])
                nc.vector.tensor_tensor(out=s0b[:, nt, :], in0=tmp[:], in1=b8_t[:],
                                        op=mybir.AluOpType.add)
        # pair view: row q = [ghat[2q] | ghat[2q+1]], 128 bf16 = 256 B
        g_pairs = g_full[:].rearrange("(q two) f -> q (two f)", two=2)

        # ---- phase 2: pair-gather + packed segmented-sum matmuls ----
        p2 = ctx.enter_context(tc.tile_pool(name="p2", bufs=2))
        psum2 = ctx.enter_context(tc.tile_pool(name="ps2", bufs=2, space="PSUM"))
        outp = ctx.enter_context(tc.tile_pool(name="outp", bufs=3))

        MSGB = 8
        wi = 0
        for si, tiles in enumerate(supers):
            gbuf = {}
            for g in range(4):
                w = windows[wi]
                wi += 1
                wch, nb, w0 = w["wch"], w["nb"], w["w0"]
                grp_, par_ = g // 2, g % 2
                idx_t = p2.tile([128, wch * 8], i16, tag="idx", bufs=MSGB)
                nc.sync.dma_start(idx_t[:], idx_ap[:, w0 // 16:(w0 + wch * 128) // 16])
                dstA_t = p2.tile([128, wch], bf16, tag="dstA", bufs=MSGB)
                nc.sync.dma_start(dstA_t[:], dstA_ap[:, w0 // 128: w0 // 128 + wch])
                msg = p2.tile([128, wch, 128], bf16, tag="msg", bufs=MSGB)
                nc.gpsimd.dma_gather(
                    msg[:], g_pairs[grp_ * GPAIRS:(grp_ + 1) * GPAIRS, :],
                    idx_t[:], wch * 128, wch * 128, 128,
                    single_packet=False, queue_num=(si + g) % 4,
                )
                ohA = p2.tile([128, wch, 128], bf16, tag="ohA", bufs=MSGB)
                nc.vector.tensor_tensor(
                    out=ohA[:],
                    in0=iota_t[:].unsqueeze(1).broadcast_to([128, wch, 128]),
                    in1=dstA_t[:].unsqueeze(2).broadcast_to([128, wch, 128]),
                    op=mybir.AluOpType.is_equal)
                ohB = None
                if nb:
                    dstB_t = p2.tile([128, nb], bf16, tag="dstB", bufs=MSGB)
                    nc.sync.dma_start(dstB_t[:], dstB_ap[:, w["boff"]: w["boff"] + nb])
                    ohB = p2.tile([128, nb, 128], bf16, tag="ohB", bufs=MSGB)
                    nc.vector.tensor_tensor(
                        out=ohB[:],
                        in0=iota_t[:].unsqueeze(1).broadcast_to([128, nb, 128]),
                        in1=dstB_t[:].unsqueeze(2).broadcast_to([128, nb, 128]),
                        op=mybir.AluOpType.is_equal)
                gbuf[g] = (msg, ohA, ohB, par_, w["bcols"])

            stg = outp.tile([128, len(tiles), OUT_FEAT], f32, tag="stg")
            for ti, t in enumerate(tiles):
                acc = psum2.tile([128, OUT_FEAT], f32, tag=f"acc{ti % 4}",
                                 name=f"acc_{si}_{ti}")
                plist = [(g, kd, ci) for g in range(4)
                         for (kd, ci) in pieces[(si, g)][t]]
                for pi, (g, kd, ci) in enumerate(plist):
                    msg, ohA, ohB, par_, bcols = gbuf[g]
                    oh = ohA if kd == "A" else ohB
                    c = ci if kd == "A" else bcols[ci]
                    nc.tensor.matmul(
                        acc[:], oh[:, ci, :] if kd == "A" else ohB[:, ci, :],
                        msg[:, c, par_ * OUT_FEAT:(par_ + 1) * OUT_FEAT],
                        start=(pi == 0), stop=(pi == len(plist) - 1),
                    )
                tmpf = outp.tile([128, OUT_FEAT], f32, tag="tmpf", bufs=4)
                nc.scalar.activation(tmpf[:], acc[:], Copy,
                                     scale=dinv_t[:, t:t + 1])
                nc.vector.tensor_tensor(out=stg[:, ti, :], in0=tmpf[:],
                                        in1=s0b[:, t, :], op=mybir.AluOpType.add)
            nc.sync.dma_start(out_ap[:, tiles[0]:tiles[0] + len(tiles), :], stg[:])

    nc.compile()
    return nc


_CACHED = {}


def _cache_key(meta):
    return (meta["S_total"], meta["CB_total"],
            tuple((w["wch"], w["nreal"], w["nb"]) for w in meta["windows"]))


def _in_maps(per_core, consts):
    maps = []
    for c in range(NCORES):
        maps.append({
            "xt_in": per_core["xT"][c],
            "idx_in": per_core["idx"][c],
            "dsta_in": per_core["dstA"][c],
            "dstb_in": per_core["dstB"][c],
            "dinv_in": per_core["dinv"][c],
            "dinv2_in": per_core["dinv2"][c],
            "w2_in": consts["W2"],
            "iota_in": consts["iota"],
            "b8_in": consts["b8"],
        })
    return maps


def kernel(x, edge_index, W_gc, b_gc, W_fc, b_fc):
    from concourse import bass_utils

    meta, per_core, consts = _preprocess(x, edge_index, W_gc, b_gc, W_fc, b_fc)
    key = _cache_key(meta)
    if key in _CACHED:
        nc = _CACHED[key]
    else:
        nc = _build(meta)
        _CACHED.clear()
        _CACHED[key] = nc

    res = bass_utils.run_bass_kernel_spmd(nc, _in_maps(per_core, consts),
                                          core_ids=list(range(NCORES)))
    out = np.empty((N_NODES, OUT_FEAT), np.float32)
    for c in range(NCORES):
        oc = res.results[c]["y_out"]                      # [128, 98, 64]
        out[c * SHARD:(c + 1) * SHARD] = (
            oc.transpose(1, 0, 2).reshape(PADSHARD, OUT_FEAT)[:SHARD])
    return out



# revision 27
# speedup vs baseline: 1.0511x; 1.0511x over previous
"""GCN encoder kernel for 8 Trainium2 NeuronCores.

Math: out = A_hat @ (x @ (W_gc @ W_fc)) + (b_gc @ W_fc + b_fc), with
A_hat = D^-1/2 (A + I) D^-1/2 (degree over destinations incl self-loops).

Factorization: out[d] = dinv[d] * (sum_{e: dst=d} ghat[src_e] + ghat[d]) + b_out
with ghat[s] = dinv[s] * (x[s] @ W2), W2 = W_gc @ W_fc. So no per-edge norm
is needed on device: the one-hot segmented-sum matmul uses exact 0/1 weights.

Distribution (1D graph parallel, dst-partitioned):
  - nodes sharded 8 ways (12544-padded); each core computes its shard of
    ghat (bf16) from host-pretransposed x, AllGather -> full bf16 table,
  - per core, edges with local destination are bucketed by
    (dst tile, src pair-group, src parity); gather elements are PAIRS of
    bf16 rows (256 B) so indices fit int16 with only 2 groups; the matmul
    rhs selects the 64-col half by parity (bucket-constant),
  - buckets are packed back-to-back (padded only to the max count over the
    8 cores, for the SPMD-common schedule); chunks that straddle a tile
    boundary get a second compacted one-hot (ohB) and an extra matmul,
  - self-loop contribution + bias is folded into s0b = dinv^2*g + b_out,
    added at flush: out = dinv*acc + s0b.
"""
import numpy as np
import ml_dtypes
from contextlib import ExitStack

N_NODES = 100000
IN_FEAT = 256
OUT_FEAT = 64
NCORES = 8
SHARD = N_NODES // NCORES          # 12500
NTILES = 98                        # ceil(12500/128)
PADSHARD = NTILES * 128            # 12544
GTAB_ROWS = NCORES * PADSHARD      # 100352
NPAIRS = GTAB_ROWS // 2            # 50176
NGROUPS = 2
GPAIRS = NPAIRS // NGROUPS         # 25088 (< 32768, int16-addressable)
SUPER = 6                          # dst tiles per super
DST_SENTINEL = 255.0               # matches no iota column -> zero one-hot row


def _preprocess(x, edge_index, W_gc, b_gc, W_fc, b_fc):
    """Host-side index/weight preprocessing. Returns the (input-derived,
    core-common) schedule and per-core device arrays."""
    x = np.asarray(x, np.float32)
    W2 = (np.asarray(W_gc, np.float64) @ np.asarray(W_fc, np.float64)).astype(np.float32)
    b_out = (np.asarray(b_gc, np.float64) @ np.asarray(W_fc, np.float64)
             + np.asarray(b_fc, np.float64)).astype(np.float32)

    src = np.asarray(edge_index[0], np.int64)
    dst = np.asarray(edge_index[1], np.int64)
    deg = np.bincount(dst, minlength=N_NODES).astype(np.float64) + 1.0
    dinv = (1.0 / np.sqrt(deg)).astype(np.float32)

    core = dst // SHARD
    dloc = dst % SHARD
    tile_id = dloc // 128
    dst_local = (dloc % 128).astype(np.float32)
    gsrc = (src // SHARD) * PADSHARD + (src % SHARD)
    pairi = gsrc >> 1
    par = (gsrc & 1).astype(np.int64)
    grp = pairi // GPAIRS
    lidx = (pairi % GPAIRS).astype(np.int16)
    gp = grp * 2 + par

    key = (core * NTILES + tile_id) * 4 + gp
    order = np.argsort(key, kind="stable")
    key_s = key[order]
    lidx_s = lidx[order]
    dstl_s = dst_local[order]
    counts = np.bincount(key_s, minlength=NCORES * NTILES * 4).reshape(NCORES, NTILES, 4)
    flat = counts.reshape(NCORES, -1)
    starts = np.zeros_like(counts)
    starts.reshape(NCORES, -1)[:, 1:] = np.cumsum(flat, axis=1)[:, :-1]
    starts += np.concatenate([[0], np.cumsum(flat.sum(axis=1))[:-1]]).reshape(-1, 1, 1)

    mx = np.maximum(counts.max(axis=0), 128)   # [NTILES, 4]; >=128 keeps the
    # "at most one tile starts per chunk" invariant of the packed layout.

    supers = [list(range(i, min(i + SUPER, NTILES))) for i in range(0, NTILES, SUPER)]

    # ---- core-common packed schedule ----
    windows = []           # per (super, gp)
    pieces = {}            # (si, gp) -> {t: [('A', wchunk) | ('B', bcol)]}
    slot_base = {}         # (si, gp, t) -> global slot start
    pos = 0
    for si, tiles in enumerate(supers):
        for g in range(4):
            w0 = pos
            bcols = []
            wp = {}
            for t in tiles:
                at = pos - w0
                slot_base[(si, g, t)] = pos
                m = int(mx[t, g])
                bt = at + m
                c0, c1 = at // 128, (bt - 1) // 128
                pl = []
                if at % 128 != 0:
                    pl.append(("B", len(bcols)))
                    bcols.append(c0)
                    c0 += 1
                pl.extend(("A", c) for c in range(c0, c1 + 1))
                wp[t] = pl
                pos += m
            nreal = pos - w0
            pos = ((pos + 127) // 128) * 128
            windows.append(dict(si=si, gp=g, w0=w0, wch=(pos - w0) // 128,
                                nreal=nreal, nb=len(bcols), bcols=bcols))
            pieces[(si, g)] = wp
    S_total = pos
    C_total = S_total // 128
    boff = 0
    for w in windows:
        w["boff"] = boff
        boff += w["nb"]
    CB_total = max(boff, 1)

    # ---- per-core slot data ----
    idx_all = np.zeros((NCORES, S_total), np.int16)
    dstA_all = np.full((NCORES, S_total), DST_SENTINEL, np.float32)
    dstB_all = np.full((NCORES, CB_total * 128), DST_SENTINEL, np.float32)
    wi = 0
    for si, tiles in enumerate(supers):
        for g in range(4):
            w = windows[wi]
            wi += 1
            wp = pieces[(si, g)]
            for t in tiles:
                a_g = slot_base[(si, g, t)]
                at = a_g - w["w0"]
                lead = (128 - at % 128) % 128
                jcol = w["boff"] + wp[t][0][1] if at % 128 != 0 else -1
                for c in range(NCORES):
                    n = int(counts[c, t, g])
                    if n == 0:
                        continue
                    s0 = int(starts[c, t, g])
                    idx_all[c, a_g:a_g + n] = lidx_s[s0:s0 + n]
                    vals = dstl_s[s0:s0 + n]
                    nlead = min(lead, n)
                    if nlead:
                        p0 = a_g % 128
                        dstB_all[c, jcol * 128 + p0: jcol * 128 + p0 + nlead] = vals[:nlead]
                    if n > nlead:
                        dstA_all[c, a_g + nlead: a_g + n] = vals[nlead:]

    # ---- device layouts ----
    idx_dev = np.ascontiguousarray(
        np.tile(idx_all.reshape(NCORES, -1, 16).transpose(0, 2, 1), (1, 8, 1)))
    dstA_dev = np.ascontiguousarray(
        dstA_all.reshape(NCORES, C_total, 128).transpose(0, 2, 1)).astype(ml_dtypes.bfloat16)
    dstB_dev = np.ascontiguousarray(
        dstB_all.reshape(NCORES, CB_total, 128).transpose(0, 2, 1)).astype(ml_dtypes.bfloat16)

    x_pad = np.zeros((NCORES, PADSHARD, IN_FEAT), np.float32)
    x_pad[:, :SHARD] = x.reshape(NCORES, SHARD, IN_FEAT)
    # [NC, NTILES, p(feat%128), k(feat//128), m(node%128)] - per-tile contiguous
    xT_dev = np.ascontiguousarray(
        x_pad.reshape(NCORES, NTILES, 128, 2, 128).transpose(0, 1, 4, 3, 2)
    ).astype(ml_dtypes.bfloat16)

    dinv_pad = np.zeros((NCORES, PADSHARD), np.float32)
    dinv_pad[:, :SHARD] = dinv.reshape(NCORES, SHARD)
    dinv_dev = np.ascontiguousarray(
        dinv_pad.reshape(NCORES, NTILES, 128).transpose(0, 2, 1))  # [NC, 128, NT]
    dinv2_dev = np.ascontiguousarray(dinv_dev * dinv_dev)

    iota_np = np.tile(np.arange(128, dtype=np.float32)[None, :], (128, 1)).astype(ml_dtypes.bfloat16)
    b8_np = np.tile(b_out[None, :], (128, 1)).astype(np.float32)

    meta = dict(supers=supers, windows=windows, pieces=pieces,
                S_total=S_total, C_total=C_total, CB_total=CB_total)
    per_core = dict(xT=xT_dev, idx=idx_dev, dstA=dstA_dev, dstB=dstB_dev,
                    dinv=dinv_dev, dinv2=dinv2_dev)
    consts = dict(W2=W2.astype(ml_dtypes.bfloat16), iota=iota_np, b8=b8_np)
    return meta, per_core, consts


def _build(meta):
    import concourse.bass as bass
    import concourse.tile as tile
    from concourse import bacc, mybir

    supers = meta["supers"]
    windows = meta["windows"]
    pieces = meta["pieces"]
    S_total = meta["S_total"]
    C_total = meta["C_total"]
    CB_total = meta["CB_total"]

    nc = bacc.Bacc("TRN2", target_bir_lowering=False, debug=False,
                   num_devices=NCORES, num_swdge_queues=4)
    f32, bf16, i16 = mybir.dt.float32, mybir.dt.bfloat16, mybir.dt.int16
    Copy = mybir.ActivationFunctionType.Copy

    xT_ap = nc.dram_tensor("xt_in", [NTILES, 128, 2, 128], bf16, kind="ExternalInput").ap()
    idx_ap = nc.dram_tensor("idx_in", [128, S_total // 16], i16, kind="ExternalInput").ap()
    dstA_ap = nc.dram_tensor("dsta_in", [128, C_total], bf16, kind="ExternalInput").ap()
    dstB_ap = nc.dram_tensor("dstb_in", [128, CB_total], bf16, kind="ExternalInput").ap()
    W2_ap = nc.dram_tensor("w2_in", [IN_FEAT, OUT_FEAT], bf16, kind="ExternalInput").ap()
    iota_ap = nc.dram_tensor("iota_in", [128, 128], bf16, kind="ExternalInput").ap()
    dinv_ap = nc.dram_tensor("dinv_in", [128, NTILES], f32, kind="ExternalInput").ap()
    dinv2_ap = nc.dram_tensor("dinv2_in", [128, NTILES], f32, kind="ExternalInput").ap()
    b8_ap = nc.dram_tensor("b8_in", [128, OUT_FEAT], f32, kind="ExternalInput").ap()
    out_ap = nc.dram_tensor("y_out", [128, NTILES, OUT_FEAT], f32, kind="ExternalOutput").ap()

    with tile.TileContext(nc) as tc, ExitStack() as ctx:
        dram = ctx.enter_context(tc.tile_pool(name="dram", bufs=1, space="DRAM"))
        g_c = dram.tile([PADSHARD, OUT_FEAT], bf16)
        g_full = dram.tile([GTAB_ROWS, OUT_FEAT], bf16)

        cpool = ctx.enter_context(tc.tile_pool(name="consts", bufs=1))
        iota_t = cpool.tile([128, 128], bf16)
        nc.sync.dma_start(iota_t[:], iota_ap[:])
        b8_t = cpool.tile([128, OUT_FEAT], f32)
        nc.sync.dma_start(b8_t[:], b8_ap[:])
        dinv_t = cpool.tile([128, NTILES], f32)
        nc.sync.dma_start(dinv_t[:], dinv_ap[:])
        dinv2_t = cpool.tile([128, NTILES], f32)
        nc.sync.dma_start(dinv2_t[:], dinv2_ap[:])
        gl_sb = cpool.tile([128, NTILES, OUT_FEAT], bf16)     # ghat local shard
        s0b = cpool.tile([128, NTILES, OUT_FEAT], bf16)       # dinv^2*g + b_out

        # ---- phase 1: ghat_c = dinv * (x_c @ W2), bf16 ----
        with tc.tile_pool(name="ph1", bufs=3) as ph1, \
             tc.tile_pool(name="ph1c", bufs=1) as ph1c, \
             tc.tile_pool(name="ph1ps", bufs=4, space="PSUM") as ph1ps:
            w2_t = ph1c.tile([128, 2, OUT_FEAT], bf16)
            nc.sync.dma_start(w2_t[:], W2_ap.rearrange("(k p) f -> p k f", p=128))
            for nt in range(NTILES):
                xt = ph1.tile([128, 2, 128], bf16, tag="xt")
                nc.sync.dma_start(xt[:], xT_ap[nt])
                gp = ph1ps.tile([128, OUT_FEAT], f32, tag="gps")
                for k in range(2):
                    nc.tensor.matmul(gp[:], xt[:, k, :], w2_t[:, k, :],
                                     start=(k == 0), stop=(k == 1))
                nc.vector.tensor_tensor(
                    out=gl_sb[:, nt, :], in0=gp[:],
                    in1=dinv_t[:, nt:nt + 1].broadcast_to([128, OUT_FEAT]),
                    op=mybir.AluOpType.mult)
                tmp = ph1.tile([128, OUT_FEAT], f32, tag="s0t")
                nc.scalar.activation(tmp[:], gp[:], Copy,
                                     scale=dinv2_t[:, nt:nt + 1])
                nc.vector.tensor_tensor(out=s0b[:, nt, :], in0=tmp[:], in1=b8_t[:],
                                        op=mybir.AluOpType.add)
                nc.sync.dma_start(g_c[nt * 128:(nt + 1) * 128, :], gl_sb[:, nt, :])

        # ---- allgather ghat (bf16) ----
        nc.gpsimd.collective_compute(
            "AllGather", mybir.AluOpType.bypass,
            ins=[g_c.opt()], outs=[g_full.opt()],
            replica_groups=[list(range(NCORES))],
        )
        # pair view: row q = [ghat[2q] | ghat[2q+1]], 128 bf16 = 256 B
        g_pairs = g_full[:].rearrange("(q two) f -> q (two f)", two=2)

        # ---- phase 2: pair-gather + packed segmented-sum matmuls ----
        p2 = ctx.enter_context(tc.tile_pool(name="p2", bufs=2))
        psum2 = ctx.enter_context(tc.tile_pool(name="ps2", bufs=2, space="PSUM"))
        outp = ctx.enter_context(tc.tile_pool(name="outp", bufs=3))

        MSGB = 8
        wi = 0
        for si, tiles in enumerate(supers):
            gbuf = {}
            for g in range(4):
                w = windows[wi]
                wi += 1
                wch, nb, w0 = w["wch"], w["nb"], w["w0"]
                grp_, par_ = g // 2, g % 2
                idx_t = p2.tile([128, wch * 8], i16, tag="idx", bufs=MSGB)
                nc.sync.dma_start(idx_t[:], idx_ap[:, w0 // 16:(w0 + wch * 128) // 16])
                dstA_t = p2.tile([128, wch], bf16, tag="dstA", bufs=MSGB)
                nc.sync.dma_start(dstA_t[:], dstA_ap[:, w0 // 128: w0 // 128 + wch])
                msg = p2.tile([128, wch, 128], bf16, tag="msg", bufs=MSGB)
                nc.gpsimd.dma_gather(
                    msg[:], g_pairs[grp_ * GPAIRS:(grp_ + 1) * GPAIRS, :],
                    idx_t[:], wch * 128, wch * 128, 128,
                    single_packet=False, queue_num=(si + g) % 4,
                )
                ohA = p2.tile([128, wch, 128], bf16, tag="ohA", bufs=MSGB)
                nc.vector.tensor_tensor(
                    out=ohA[:],
                    in0=iota_t[:].unsqueeze(1).broadcast_to([128, wch, 128]),
                    in1=dstA_t[:].unsqueeze(2).broadcast_to([128, wch, 128]),
                    op=mybir.AluOpType.is_equal)
                ohB = None
                if nb:
                    dstB_t = p2.tile([128, nb], bf16, tag="dstB", bufs=MSGB)
                    nc.sync.dma_start(dstB_t[:], dstB_ap[:, w["boff"]: w["boff"] + nb])
                    ohB = p2.tile([128, nb, 128], bf16, tag="ohB", bufs=MSGB)
                    nc.vector.tensor_tensor(
                        out=ohB[:],
                        in0=iota_t[:].unsqueeze(1).broadcast_to([128, nb, 128]),
                        in1=dstB_t[:].unsqueeze(2).broadcast_to([128, nb, 128]),
                        op=mybir.AluOpType.is_equal)
                gbuf[g] = (msg, ohA, ohB, par_, w["bcols"])

            stg = outp.tile([128, len(tiles), OUT_FEAT], f32, tag="stg")
            for ti, t in enumerate(tiles):
                acc = psum2.tile([128, OUT_FEAT], f32, tag=f"acc{ti % 4}",
                                 name=f"acc_{si}_{ti}")
                plist = [(g, kd, ci) for g in range(4)
                         for (kd, ci) in pieces[(si, g)][t]]
                for pi, (g, kd, ci) in enumerate(plist):
                    msg, ohA, ohB, par_, bcols = gbuf[g]
                    oh = ohA if kd == "A" else ohB
                    c = ci if kd == "A" else bcols[ci]
                    nc.tensor.matmul(
                        acc[:], oh[:, ci, :] if kd == "A" else ohB[:, ci, :],
                        msg[:, c, par_ * OUT_FEAT:(par_ + 1) * OUT_FEAT],
                        start=(pi == 0), stop=(pi == len(plist) - 1),
                    )
                tmpf = outp.tile([128, OUT_FEAT], f32, tag="tmpf", bufs=4)
                nc.scalar.activation(tmpf[:], acc[:], Copy,
                                     scale=dinv_t[:, t:t + 1])
                nc.vector.tensor_tensor(out=stg[:, ti, :], in0=tmpf[:],
                                        in1=s0b[:, t, :], op=mybir.AluOpType.add)
            nc.sync.dma_start(out_ap[:, tiles[0]:tiles[0] + len(tiles), :], stg[:])

    nc.compile()
    return nc


_CACHED = {}


def _cache_key(meta):
    return (meta["S_total"], meta["CB_total"],
            tuple((w["wch"], w["nreal"], w["nb"]) for w in meta["windows"]))


def _in_maps(per_core, consts):
    maps = []
    for c in range(NCORES):
        maps.append({
            "xt_in": per_core["xT"][c],
            "idx_in": per_core["idx"][c],
            "dsta_in": per_core["dstA"][c],
            "dstb_in": per_core["dstB"][c],
            "dinv_in": per_core["dinv"][c],
            "dinv2_in": per_core["dinv2"][c],
            "w2_in": consts["W2"],
            "iota_in": consts["iota"],
            "b8_in": consts["b8"],
        })
    return maps


def kernel(x, edge_index, W_gc, b_gc, W_fc, b_fc):
    from concourse import bass_utils

    meta, per_core, consts = _preprocess(x, edge_index, W_gc, b_gc, W_fc, b_fc)
    key = _cache_key(meta)
    if key in _CACHED:
        nc = _CACHED[key]
    else:
        nc = _build(meta)
        _CACHED.clear()
        _CACHED[key] = nc

    res = bass_utils.run_bass_kernel_spmd(nc, _in_maps(per_core, consts),
                                          core_ids=list(range(NCORES)))
    out = np.empty((N_NODES, OUT_FEAT), np.float32)
    for c in range(NCORES):
        oc = res.results[c]["y_out"]                      # [128, 98, 64]
        out[c * SHARD:(c + 1) * SHARD] = (
            oc.transpose(1, 0, 2).reshape(PADSHARD, OUT_FEAT)[:SHARD])
    return out



# revision 28
# speedup vs baseline: 1.1335x; 1.0784x over previous
"""GCN encoder kernel for 8 Trainium2 NeuronCores.

Math: out = A_hat @ (x @ (W_gc @ W_fc)) + (b_gc @ W_fc + b_fc), with
A_hat = D^-1/2 (A + I) D^-1/2 (degree over destinations incl self-loops).

Factorization: out[d] = dinv[d] * (sum_{e: dst=d} ghat[src_e] + ghat[d]) + b_out
with ghat[s] = dinv[s] * (x[s] @ W2), W2 = W_gc @ W_fc. So no per-edge norm
is needed on device: the one-hot segmented-sum matmul uses exact 0/1 weights.

Distribution (1D graph parallel, dst-partitioned):
  - nodes sharded 8 ways (12544-padded); each core computes its shard of
    ghat (bf16) from host-pretransposed x, AllGather -> full bf16 table,
  - per core, edges with local destination are bucketed by
    (dst tile, src pair-group, src parity); gather elements are PAIRS of
    bf16 rows (256 B) so indices fit int16 with only 2 groups; the matmul
    rhs selects the 64-col half by parity (bucket-constant),
  - buckets are packed back-to-back (padded only to the max count over the
    8 cores, for the SPMD-common schedule); chunks that straddle a tile
    boundary get a second compacted one-hot (ohB) and an extra matmul,
  - self-loop contribution + bias is folded into s0b = dinv^2*g + b_out,
    added at flush: out = dinv*acc + s0b.
"""
import numpy as np
import ml_dtypes
from contextlib import ExitStack

N_NODES = 100000
IN_FEAT = 256
OUT_FEAT = 64
NCORES = 8
SHARD = N_NODES // NCORES          # 12500
NTILES = 98                        # ceil(12500/128)
PADSHARD = NTILES * 128            # 12544
GTAB_ROWS = NCORES * PADSHARD      # 100352
NPAIRS = GTAB_ROWS // 2            # 50176
NGROUPS = 2
GPAIRS = NPAIRS // NGROUPS         # 25088 (< 32768, int16-addressable)
SUPER = 6                          # dst tiles per super
DST_SENTINEL = 255.0               # matches no iota column -> zero one-hot row


def _preprocess(x, edge_index, W_gc, b_gc, W_fc, b_fc):
    """Host-side index/weight preprocessing. Returns the (input-derived,
    core-common) schedule and per-core device arrays."""
    x = np.asarray(x, np.float32)
    W2 = (np.asarray(W_gc, np.float64) @ np.asarray(W_fc, np.float64)).astype(np.float32)
    b_out = (np.asarray(b_gc, np.float64) @ np.asarray(W_fc, np.float64)
             + np.asarray(b_fc, np.float64)).astype(np.float32)

    src = np.asarray(edge_index[0], np.int64)
    dst = np.asarray(edge_index[1], np.int64)
    deg = np.bincount(dst, minlength=N_NODES).astype(np.float64) + 1.0
    dinv = (1.0 / np.sqrt(deg)).astype(np.float32)

    core = dst // SHARD
    dloc = dst % SHARD
    tile_id = dloc // 128
    dst_local = (dloc % 128).astype(np.float32)
    gsrc = (src // SHARD) * PADSHARD + (src % SHARD)
    pairi = gsrc >> 1
    par = (gsrc & 1).astype(np.int64)
    grp = pairi // GPAIRS
    lidx = (pairi % GPAIRS).astype(np.int16)
    gp = grp * 2 + par

    key = (core * NTILES + tile_id) * 4 + gp
    order = np.argsort(key, kind="stable")
    key_s = key[order]
    lidx_s = lidx[order]
    dstl_s = dst_local[order]
    counts = np.bincount(key_s, minlength=NCORES * NTILES * 4).reshape(NCORES, NTILES, 4)
    flat = counts.reshape(NCORES, -1)
    starts = np.zeros_like(counts)
    starts.reshape(NCORES, -1)[:, 1:] = np.cumsum(flat, axis=1)[:, :-1]
    starts += np.concatenate([[0], np.cumsum(flat.sum(axis=1))[:-1]]).reshape(-1, 1, 1)

    mx = np.maximum(counts.max(axis=0), 128)   # [NTILES, 4]; >=128 keeps the
    # "at most one tile starts per chunk" invariant of the packed layout.

    supers = [list(range(i, min(i + SUPER, NTILES))) for i in range(0, NTILES, SUPER)]

    # ---- core-common packed schedule ----
    windows = []           # per (super, gp)
    pieces = {}            # (si, gp) -> {t: [('A', wchunk) | ('B', bcol)]}
    slot_base = {}         # (si, gp, t) -> global slot start
    pos = 0
    for si, tiles in enumerate(supers):
        for g in range(4):
            w0 = pos
            bcols = []
            wp = {}
            for t in tiles:
                at = pos - w0
                slot_base[(si, g, t)] = pos
                m = int(mx[t, g])
                bt = at + m
                c0, c1 = at // 128, (bt - 1) // 128
                pl = []
                if at % 128 != 0:
                    pl.append(("B", len(bcols)))
                    bcols.append(c0)
                    c0 += 1
                pl.extend(("A", c) for c in range(c0, c1 + 1))
                wp[t] = pl
                pos += m
            nreal = pos - w0
            pos = ((pos + 127) // 128) * 128
            windows.append(dict(si=si, gp=g, w0=w0, wch=(pos - w0) // 128,
                                nreal=nreal, nb=len(bcols), bcols=bcols))
            pieces[(si, g)] = wp
    S_total = pos
    C_total = S_total // 128
    boff = 0
    for w in windows:
        w["boff"] = boff
        boff += w["nb"]
    CB_total = max(boff, 1)

    # ---- per-core slot data ----
    idx_all = np.zeros((NCORES, S_total), np.int16)
    dstA_all = np.full((NCORES, S_total), DST_SENTINEL, np.float32)
    dstB_all = np.full((NCORES, CB_total * 128), DST_SENTINEL, np.float32)
    wi = 0
    for si, tiles in enumerate(supers):
        for g in range(4):
            w = windows[wi]
            wi += 1
            wp = pieces[(si, g)]
            for t in tiles:
                a_g = slot_base[(si, g, t)]
                at = a_g - w["w0"]
                lead = (128 - at % 128) % 128
                jcol = w["boff"] + wp[t][0][1] if at % 128 != 0 else -1
                for c in range(NCORES):
                    n = int(counts[c, t, g])
                    if n == 0:
                        continue
                    s0 = int(starts[c, t, g])
                    idx_all[c, a_g:a_g + n] = lidx_s[s0:s0 + n]
                    vals = dstl_s[s0:s0 + n]
                    nlead = min(lead, n)
                    if nlead:
                        p0 = a_g % 128
                        dstB_all[c, jcol * 128 + p0: jcol * 128 + p0 + nlead] = vals[:nlead]
                    if n > nlead:
                        dstA_all[c, a_g + nlead: a_g + n] = vals[nlead:]

    # ---- device layouts ----
    idx_dev = np.ascontiguousarray(
        np.tile(idx_all.reshape(NCORES, -1, 16).transpose(0, 2, 1), (1, 8, 1)))
    dstA_dev = np.ascontiguousarray(
        dstA_all.reshape(NCORES, C_total, 128).transpose(0, 2, 1)).astype(ml_dtypes.bfloat16)
    dstB_dev = np.ascontiguousarray(
        dstB_all.reshape(NCORES, CB_total, 128).transpose(0, 2, 1)).astype(ml_dtypes.bfloat16)

    x_pad = np.zeros((NCORES, PADSHARD, IN_FEAT), np.float32)
    x_pad[:, :SHARD] = x.reshape(NCORES, SHARD, IN_FEAT)
    # [NC, NTILES, p(feat%128), k(feat//128), m(node%128)] - per-tile contiguous
    xT_dev = np.ascontiguousarray(
        x_pad.reshape(NCORES, NTILES, 128, 2, 128).transpose(0, 1, 4, 3, 2)
    ).astype(ml_dtypes.bfloat16)

    dinv_pad = np.zeros((NCORES, PADSHARD), np.float32)
    dinv_pad[:, :SHARD] = dinv.reshape(NCORES, SHARD)
    dinv_dev = np.ascontiguousarray(
        dinv_pad.reshape(NCORES, NTILES, 128).transpose(0, 2, 1))  # [NC, 128, NT]
    dinv2_dev = np.ascontiguousarray(dinv_dev * dinv_dev)

    iota_np = np.tile(np.arange(128, dtype=np.float32)[None, :], (128, 1)).astype(ml_dtypes.bfloat16)
    b8_np = np.tile(b_out[None, :], (128, 1)).astype(np.float32)

    meta = dict(supers=supers, windows=windows, pieces=pieces,
                S_total=S_total, C_total=C_total, CB_total=CB_total)
    per_core = dict(xT=xT_dev, idx=idx_dev, dstA=dstA_dev, dstB=dstB_dev,
                    dinv=dinv_dev, dinv2=dinv2_dev)
    consts = dict(W2=W2.astype(ml_dtypes.bfloat16), iota=iota_np, b8=b8_np)
    return meta, per_core, consts


def _build(meta):
    import concourse.bass as bass
    import concourse.tile as tile
    from concourse import bacc, mybir

    supers = meta["supers"]
    windows = meta["windows"]
    pieces = meta["pieces"]
    S_total = meta["S_total"]
    C_total = meta["C_total"]
    CB_total = meta["CB_total"]

    nc = bacc.Bacc("TRN2", target_bir_lowering=False, debug=False,
                   num_devices=NCORES, num_swdge_queues=4)
    f32, bf16, i16 = mybir.dt.float32, mybir.dt.bfloat16, mybir.dt.int16
    Copy = mybir.ActivationFunctionType.Copy

    xT_ap = nc.dram_tensor("xt_in", [NTILES, 128, 2, 128], bf16, kind="ExternalInput").ap()
    idx_ap = nc.dram_tensor("idx_in", [128, S_total // 16], i16, kind="ExternalInput").ap()
    dstA_ap = nc.dram_tensor("dsta_in", [128, C_total], bf16, kind="ExternalInput").ap()
    dstB_ap = nc.dram_tensor("dstb_in", [128, CB_total], bf16, kind="ExternalInput").ap()
    W2_ap = nc.dram_tensor("w2_in", [IN_FEAT, OUT_FEAT], bf16, kind="ExternalInput").ap()
    iota_ap = nc.dram_tensor("iota_in", [128, 128], bf16, kind="ExternalInput").ap()
    dinv_ap = nc.dram_tensor("dinv_in", [128, NTILES], f32, kind="ExternalInput").ap()
    dinv2_ap = nc.dram_tensor("dinv2_in", [128, NTILES], f32, kind="ExternalInput").ap()
    b8_ap = nc.dram_tensor("b8_in", [128, OUT_FEAT], f32, kind="ExternalInput").ap()
    out_ap = nc.dram_tensor("y_out", [128, NTILES, OUT_FEAT], f32, kind="ExternalOutput").ap()

    with tile.TileContext(nc) as tc, ExitStack() as ctx:
        dram = ctx.enter_context(tc.tile_pool(name="dram", bufs=1, space="DRAM"))
        g_c = dram.tile([PADSHARD, OUT_FEAT], bf16)
        g_full = dram.tile([GTAB_ROWS, OUT_FEAT], bf16)

        cpool = ctx.enter_context(tc.tile_pool(name="consts", bufs=1))
        iota_t = cpool.tile([128, 128], bf16)
        nc.sync.dma_start(iota_t[:], iota_ap[:])
        b8_t = cpool.tile([128, OUT_FEAT], f32)
        nc.sync.dma_start(b8_t[:], b8_ap[:])
        dinv_t = cpool.tile([128, NTILES], f32)
        nc.sync.dma_start(dinv_t[:], dinv_ap[:])
        dinv2_t = cpool.tile([128, NTILES], f32)
        nc.sync.dma_start(dinv2_t[:], dinv2_ap[:])
        gl_sb = cpool.tile([128, NTILES, OUT_FEAT], bf16)     # ghat local shard
        s0b = cpool.tile([128, NTILES, OUT_FEAT], bf16)       # dinv^2*g + b_out

        # ---- phase 1: ghat_c = dinv * (x_c @ W2), bf16 ----
        with tc.tile_pool(name="ph1", bufs=3) as ph1, \
             tc.tile_pool(name="ph1c", bufs=1) as ph1c, \
             tc.tile_pool(name="ph1ps", bufs=4, space="PSUM") as ph1ps:
            w2_t = ph1c.tile([128, 2, OUT_FEAT], bf16)
            nc.sync.dma_start(w2_t[:], W2_ap.rearrange("(k p) f -> p k f", p=128))
            for nt in range(NTILES):
                xt = ph1.tile([128, 2, 128], bf16, tag="xt")
                nc.sync.dma_start(xt[:], xT_ap[nt])
                gp = ph1ps.tile([128, OUT_FEAT], f32, tag="gps")
                for k in range(2):
                    nc.tensor.matmul(gp[:], xt[:, k, :], w2_t[:, k, :],
                                     start=(k == 0), stop=(k == 1))
                nc.vector.tensor_tensor(
                    out=gl_sb[:, nt, :], in0=gp[:],
                    in1=dinv_t[:, nt:nt + 1].broadcast_to([128, OUT_FEAT]),
                    op=mybir.AluOpType.mult)
                nc.sync.dma_start(g_c[nt * 128:(nt + 1) * 128, :], gl_sb[:, nt, :])

        # ---- allgather ghat (bf16) ----
        nc.gpsimd.collective_compute(
            "AllGather", mybir.AluOpType.bypass,
            ins=[g_c.opt()], outs=[g_full.opt()],
            replica_groups=[list(range(NCORES))],
        )
        with tc.tile_pool(name="s0p", bufs=4) as s0p:
            for nt in range(NTILES):
                tmp = s0p.tile([128, OUT_FEAT], f32, tag="s0t")
                nc.scalar.activation(tmp[:], gl_sb[:, nt, :], Copy,
                                     scale=dinv_t[:, nt:nt + 1])
                nc.vector.tensor_tensor(out=s0b[:, nt, :], in0=tmp[:], in1=b8_t[:],
                                        op=mybir.AluOpType.add)
        # pair view: row q = [ghat[2q] | ghat[2q+1]], 128 bf16 = 256 B
        g_pairs = g_full[:].rearrange("(q two) f -> q (two f)", two=2)

        # ---- phase 2: pair-gather + packed segmented-sum matmuls ----
        p2 = ctx.enter_context(tc.tile_pool(name="p2", bufs=2))
        psum2 = ctx.enter_context(tc.tile_pool(name="ps2", bufs=2, space="PSUM"))
        outp = ctx.enter_context(tc.tile_pool(name="outp", bufs=3))

        MSGB = 8
        wi = 0
        for si, tiles in enumerate(supers):
            gbuf = {}
            for g in range(4):
                w = windows[wi]
                wi += 1
                wch, nb, w0 = w["wch"], w["nb"], w["w0"]
                grp_, par_ = g // 2, g % 2
                idx_t = p2.tile([128, wch * 8], i16, tag="idx", bufs=MSGB)
                nc.sync.dma_start(idx_t[:], idx_ap[:, w0 // 16:(w0 + wch * 128) // 16])
                dstA_t = p2.tile([128, wch], bf16, tag="dstA", bufs=MSGB)
                nc.sync.dma_start(dstA_t[:], dstA_ap[:, w0 // 128: w0 // 128 + wch])
                msg = p2.tile([128, wch, 128], bf16, tag="msg", bufs=MSGB)
                nc.gpsimd.dma_gather(
                    msg[:], g_pairs[grp_ * GPAIRS:(grp_ + 1) * GPAIRS, :],
                    idx_t[:], wch * 128, wch * 128, 128,
                    single_packet=False, queue_num=(si + g) % 4,
                )
                ohA = p2.tile([128, wch, 128], bf16, tag="ohA", bufs=MSGB)
                nc.vector.tensor_tensor(
                    out=ohA[:],
                    in0=iota_t[:].unsqueeze(1).broadcast_to([128, wch, 128]),
                    in1=dstA_t[:].unsqueeze(2).broadcast_to([128, wch, 128]),
                    op=mybir.AluOpType.is_equal)
                ohB = None
                if nb:
                    dstB_t = p2.tile([128, nb], bf16, tag="dstB", bufs=MSGB)
                    nc.sync.dma_start(dstB_t[:], dstB_ap[:, w["boff"]: w["boff"] + nb])
                    ohB = p2.tile([128, nb, 128], bf16, tag="ohB", bufs=MSGB)
                    nc.vector.tensor_tensor(
                        out=ohB[:],
                        in0=iota_t[:].unsqueeze(1).broadcast_to([128, nb, 128]),
                        in1=dstB_t[:].unsqueeze(2).broadcast_to([128, nb, 128]),
                        op=mybir.AluOpType.is_equal)
                gbuf[g] = (msg, ohA, ohB, par_, w["bcols"])

            stg = outp.tile([128, len(tiles), OUT_FEAT], f32, tag="stg")
            for ti, t in enumerate(tiles):
                acc = psum2.tile([128, OUT_FEAT], f32, tag=f"acc{ti % 4}",
                                 name=f"acc_{si}_{ti}")
                plist = [(g, kd, ci) for g in range(4)
                         for (kd, ci) in pieces[(si, g)][t]]
                for pi, (g, kd, ci) in enumerate(plist):
                    msg, ohA, ohB, par_, bcols = gbuf[g]
                    oh = ohA if kd == "A" else ohB
                    c = ci if kd == "A" else bcols[ci]
                    nc.tensor.matmul(
                        acc[:], oh[:, ci, :] if kd == "A" else ohB[:, ci, :],
                        msg[:, c, par_ * OUT_FEAT:(par_ + 1) * OUT_FEAT],
                        start=(pi == 0), stop=(pi == len(plist) - 1),
                    )
                tmpf = outp.tile([128, OUT_FEAT], f32, tag="tmpf", bufs=4)
                nc.scalar.activation(tmpf[:], acc[:], Copy,
                                     scale=dinv_t[:, t:t + 1])
                nc.vector.tensor_tensor(out=stg[:, ti, :], in0=tmpf[:],
                                        in1=s0b[:, t, :], op=mybir.AluOpType.add)
            nc.sync.dma_start(out_ap[:, tiles[0]:tiles[0] + len(tiles), :], stg[:])

    nc.compile()
    return nc


_CACHED = {}


def _cache_key(meta):
    return (meta["S_total"], meta["CB_total"],
            tuple((w["wch"], w["nreal"], w["nb"]) for w in meta["windows"]))


def _in_maps(per_core, consts):
    maps = []
    for c in range(NCORES):
        maps.append({
            "xt_in": per_core["xT"][c],
            "idx_in": per_core["idx"][c],
            "dsta_in": per_core["dstA"][c],
            "dstb_in": per_core["dstB"][c],
            "dinv_in": per_core["dinv"][c],
            "dinv2_in": per_core["dinv2"][c],
            "w2_in": consts["W2"],
            "iota_in": consts["iota"],
            "b8_in": consts["b8"],
        })
    return maps


def kernel(x, edge_index, W_gc, b_gc, W_fc, b_fc):
    from concourse import bass_utils

    meta, per_core, consts = _preprocess(x, edge_index, W_gc, b_gc, W_fc, b_fc)
    key = _cache_key(meta)
    if key in _CACHED:
        nc = _CACHED[key]
    else:
        nc = _build(meta)
        _CACHED.clear()
        _CACHED[key] = nc

    res = bass_utils.run_bass_kernel_spmd(nc, _in_maps(per_core, consts),
                                          core_ids=list(range(NCORES)))
    out = np.empty((N_NODES, OUT_FEAT), np.float32)
    for c in range(NCORES):
        oc = res.results[c]["y_out"]                      # [128, 98, 64]
        out[c * SHARD:(c + 1) * SHARD] = (
            oc.transpose(1, 0, 2).reshape(PADSHARD, OUT_FEAT)[:SHARD])
    return out

